# revision 1
# baseline (speedup 1.0000x reference)
"""Fused multi-head causal+padding attention for Trainium2 (Bass/Tile).

Problem: nn_Attention (B=8, T=1024, C=512, H=8, D=64, TT=4), f32.
Sharding: data-parallel over batch B across 8 NeuronCores (1 batch elem/core).

Per-core pipeline (batch b, everything stays on-chip between HBM load/store):
  1. x^T via PE transposes (needed so q/k come out in [d, t] layout).
  2. qk^T = W_qk^T @ x^T  (heads in [d, t] layout -> no transposes in attention)
     v    = x @ W_v       (standard [t, d] layout -> AV lhsT)
  3. per head: S^T[k,q] = k^T.T@q^T ; exp on ACT; multiplicative mask (bf16);
     y^T[d,q] (+ denominator row via an appended ones column on v) on PE;
     per-column normalize via reciprocal + partition_broadcast.
  4. out = y @ W_proj + b_eff (bias preloaded into PSUM via a K=1 matmul).

Host-side prep folds the 1/sqrt(D) scale into W_q/b_q and folds
b_v @ W_proj + b_proj into a single effective output bias.
"""

import numpy as np
import ml_dtypes
from contextlib import ExitStack

B, T, C, H, TT = 8, 1024, 512, 8, 4
D = C // H
NCORES = 8

_CACHE = {}


def _build_nc(reps=1, upto=4):
    import concourse.bass as bass
    import concourse.mybir as mybir
    import concourse.tile as tile
    from concourse import bacc
    from concourse.bass import ts
    from concourse.masks import make_identity

    dt = mybir.dt
    F32, F32R, BF16 = dt.float32, dt.float32r, dt.bfloat16
    AF = mybir.ActivationFunctionType

    nc = bacc.Bacc("TRN2", target_bir_lowering=False, debug=False,
                   num_devices=NCORES)

    x_d = nc.dram_tensor("x", [T, C], F32, kind="ExternalInput").ap()
    wqk_d = nc.dram_tensor("wqk", [C, 2 * C], F32R, kind="ExternalInput").ap()
    wv_d = nc.dram_tensor("wv", [C, C], F32R, kind="ExternalInput").ap()
    wp_d = nc.dram_tensor("wp", [C, C], F32R, kind="ExternalInput").ap()
    bqk_d = nc.dram_tensor("bqk", [2 * C], F32, kind="ExternalInput").ap()
    beff_d = nc.dram_tensor("beff", [1, C], F32R, kind="ExternalInput").ap()
    ones_d = nc.dram_tensor("ones1", [1, 128], F32R, kind="ExternalInput").ap()
    mask_d = nc.dram_tensor("maskT", [T, T], BF16, kind="ExternalInput").ap()
    out_d = nc.dram_tensor("out", [T, C], F32, kind="ExternalOutput").ap()

    TK = T // 128   # 8 tiles of 128 along t
    CK = C // 128   # 4 tiles of 128 along c

    with tile.TileContext(nc) as tc, ExitStack() as ctx:
        consts = ctx.enter_context(tc.tile_pool(name="consts", bufs=1))

        ident = consts.tile([128, 128], F32)
        make_identity(nc, ident)
        # dummy exp: pulls the ~2.7us ACT exp-table load into the DMA-bound
        # front (ACT idle here) instead of the first real exp in attention
        warm = consts.tile([1, 128], F32)
        nc.scalar.activation(warm, ident[0:1, :], AF.Exp)
        # weights on the scalar-engine HWDGE queue so the x loads (sync
        # queue) land first and compute starts immediately; mask on SWDGE.
        wqk_s = consts.tile([128, CK, 2 * C], F32R)
        nc.scalar.dma_start(out=wqk_s, in_=wqk_d.rearrange("(j p) n -> p j n", p=128))
        wv_s = consts.tile([128, CK, C], F32R)
        nc.scalar.dma_start(out=wv_s, in_=wv_d.rearrange("(j p) n -> p j n", p=128))
        wp_s = consts.tile([128, CK, C], F32R)
        nc.scalar.dma_start(out=wp_s, in_=wp_d.rearrange("(j p) n -> p j n", p=128))
        bqk_s = consts.tile([128, 2 * C // 128], F32)
        nc.gpsimd.dma_start(out=bqk_s, in_=bqk_d.rearrange("(i p) -> p i", p=128))
        beff_s = consts.tile([1, C], F32R)
        nc.scalar.dma_start(out=beff_s, in_=beff_d)
        ones1 = consts.tile([1, 128], F32R)
        nc.scalar.dma_start(out=ones1, in_=ones_d)
        mask_s = consts.tile([128, TK, T], BF16)
        nc.gpsimd.dma_start(out=mask_s, in_=mask_d.rearrange("(kt p) q -> p kt q", p=128))

        xT = consts.tile([128, CK, T], F32R)
        qkT = consts.tile([128, 2 * C // 128, T], F32R)
        vaug = consts.tile([128, TK, H, D + 1], BF16)
        yT = consts.tile([128, CK, T], F32R)

        def run_body():
            body(nc, tc, ts, F32, F32R, BF16, AF, TK, CK,
                 x_d, out_d, ident, wqk_s, wv_s, wp_s, bqk_s, beff_s, ones1,
                 mask_s, xT, qkT, vaug, yT, upto)

        if reps == 1:
            run_body()
        else:
            with tc.For_i(0, reps, 1):
                run_body()

    nc.compile()
    return nc


def body(nc, tc, ts, F32, F32R, BF16, AF, TK, CK,
         x_d, out_d, ident, wqk_s, wv_s, wp_s, bqk_s, beff_s, ones1,
         mask_s, xT, qkT, vaug, yT, upto=4):
        # ---- phase 1: load x, transpose to x^T ----
        with tc.tile_pool(name="xstage", bufs=3) as xst, \
             tc.tile_pool(name="ptr", bufs=4, space="PSUM") as ptr:
            for i in range(TK):
                xs = xst.tile([128, C], F32)
                nc.sync.dma_start(out=xs, in_=x_d[ts(i, 128), :])
                for j in range(CK):
                    pt = ptr.tile([128, 128], F32)
                    nc.tensor.transpose(pt, xs[:, ts(j, 128)], ident)
                    nc.vector.tensor_copy(xT[:, j, ts(i, 128)], pt)

        # ---- phase 2: qk^T (transposed) and v (standard, with ones col) ----
        # Emission order interleaves q/k tile pairs (head-pair h//2 needs
        # qkT tiles i and 4+i) with v t-tiles so head-0 attention can start
        # after ~1/4 of this phase instead of all of it.
        nc.gpsimd.memset(vaug[:, :, :, D:D + 1], 1.0)
        with tc.tile_pool(name="pqk", bufs=4, space="PSUM") as pqk:

            def qk_tile(i):
                for n in range(T // 512):          # 512-chunk of t
                    ps = pqk.tile([128, 512], F32)
                    for j in range(CK):
                        nc.tensor.matmul(
                            ps, wqk_s[:, j, ts(i, 128)],
                            xT[:, j, ts(n, 512)],
                            start=(j == 0), stop=(j == CK - 1))
                    nc.vector.tensor_scalar_add(qkT[:, i, ts(n, 512)], ps,
                                                bqk_s[:, i:i + 1])

            def v_tile(i):
                ps = pqk.tile([128, 512], F32)
                for j in range(CK):
                    nc.tensor.matmul(
                        ps, xT[:, j, ts(i, 128)], wv_s[:, j, :],
                        start=(j == 0), stop=(j == CK - 1))
                nc.scalar.activation(
                    vaug[:, i, :, 0:D],
                    ps.rearrange("p (h d) -> p h d", h=H), AF.Copy)

            for i in range(2 * C // 128):
                qk_tile(i)
            for i in range(TK):
                v_tile(i)

        # ---- phase 3: attention per head ----
        if upto < 3:
            return
        with tc.tile_pool(name="ps_s", bufs=2, space="PSUM") as ps_s, \
             tc.tile_pool(name="ps_y", bufs=2, space="PSUM") as ps_y, \
             tc.tile_pool(name="expp", bufs=4) as expp, \
             tc.tile_pool(name="rp", bufs=2) as rp, \
             tc.tile_pool(name="rbp", bufs=2) as rbp:
            for h in range(H):
                po = (h % 2) * 64
                qt = h // 2
                kt_ = C // 128 + h // 2
                y_ps = ps_y.tile([D + 1, T], F32)
                for kt in range(TK):
                    s_ps = ps_s.tile([128, T], F32)
                    for n in range(T // 512):
                        nc.tensor.matmul(
                            s_ps[:, ts(n, 512)],
                            qkT[po:po + D, kt_, ts(kt, 128)],
                            qkT[po:po + D, qt, ts(n, 512)],
                            start=True, stop=True)
                    et = expp.tile([128, T], BF16)
                    nc.scalar.activation(et, s_ps, AF.Exp)
                    # columns q >= 128*(kt+1)-1 are fully unmasked for this
                    # k-tile (causal boundary passed), so only multiply the
                    # masked prefix
                    mw = min(T, 128 * (kt + 1))
                    nc.vector.tensor_mul(et[:, :mw], et[:, :mw],
                                         mask_s[:, kt, :mw])
                    for n in range(T // 512):
                        nc.tensor.matmul(
                            y_ps[:, ts(n, 512)], vaug[:, kt, h, :],
                            et[:, ts(n, 512)],
                            start=(kt == 0), stop=(kt == TK - 1))
                rec = rp.tile([1, T], F32)
                nc.vector.reciprocal(rec, y_ps[D:D + 1, :])
                rb = rbp.tile([D, T], F32)
                nc.gpsimd.partition_broadcast(rb, rec)
                nc.vector.tensor_mul(yT[po:po + D, qt, :], y_ps[0:D, :], rb)

        # ---- phase 4: out = y @ W_proj + b_eff ----
        if upto < 4:
            return
        with tc.tile_pool(name="pp", bufs=2, space="PSUM") as pp, \
             tc.tile_pool(name="outst", bufs=3) as outst:
            for i in range(TK):
                ps = pp.tile([128, C], F32)
                nc.tensor.matmul(ps, ones1, beff_s,
                                 start=True, stop=False)
                for j in range(CK):
                    nc.tensor.matmul(ps, yT[:, j, ts(i, 128)],
                                     wp_s[:, j, :],
                                     start=False, stop=(j == CK - 1))
                ot = outst.tile([128, C], F32)
                nc.scalar.copy(ot, ps)
                nc.sync.dma_start(out=out_d[ts(i, 128), :], in_=ot)


def get_nc(reps=1, upto=4):
    key = ("nc", reps, upto)
    if key not in _CACHE:
        _CACHE[key] = _build_nc(reps, upto)
    return _CACHE[key]


def tf32_round(a):
    """Round-to-nearest-even to tf32 (10-bit mantissa). fp32r operands must be
    pre-rounded: the BIR verifier requires every producer of fp32r-matmul
    operands to emit rounded values, and DMA can't convert."""
    a = np.ascontiguousarray(a, np.float32)
    b = a.view(np.uint32)
    lsb = (b >> np.uint32(13)) & np.uint32(1)
    r = b + np.uint32(0x0FFF) + lsb
    return ((r >> np.uint32(13)) << np.uint32(13)).view(np.float32)


def make_in_maps(x, padding_mask, W_qkv, b_qkv, W_proj, b_proj):
    x = np.asarray(x, np.float32)
    padding_mask = np.asarray(padding_mask, bool)
    W_qkv = np.asarray(W_qkv, np.float32)
    b_qkv = np.asarray(b_qkv, np.float32)
    W_proj = np.asarray(W_proj, np.float32)
    b_proj = np.asarray(b_proj, np.float32)

    scale = np.float32(1.0 / np.sqrt(D))
    wqk = np.concatenate([W_qkv[:, :C] * scale, W_qkv[:, C:2 * C]], axis=1)
    wqk = tf32_round(wqk)
    wv = tf32_round(W_qkv[:, 2 * C:])
    wp = tf32_round(W_proj)
    bqk = np.concatenate([b_qkv[:C] * scale, b_qkv[C:2 * C]]).astype(np.float32)
    beff = tf32_round((b_qkv[2 * C:] @ W_proj + b_proj).reshape(1, C))

    kidx = np.arange(T, dtype=np.int32)[:, None]
    qidx = np.arange(T, dtype=np.int32)[None, :]
    causalT = kidx <= qidx                      # [k, q]
    maskT = (causalT[None] | padding_mask[:, None, :])  # [TT, k, q]
    maskT = maskT.astype(ml_dtypes.bfloat16)

    in_maps = []
    for b in range(B):
        in_maps.append({
            "x": np.ascontiguousarray(x[b]),
            "maskT": np.ascontiguousarray(maskT[b % TT]),
            "wqk": wqk, "wv": wv, "wp": wp,
            "bqk": bqk, "beff": beff,
            "ones1": np.ones((1, 128), np.float32),
        })
    return in_maps


def kernel(x, padding_mask, W_qkv, b_qkv, W_proj, b_proj):
    from concourse.bass_utils import run_bass_kernel_spmd

    nc = get_nc()
    in_maps = make_in_maps(x, padding_mask, W_qkv, b_qkv, W_proj, b_proj)
    res = run_bass_kernel_spmd(nc, in_maps, list(range(NCORES)))
    out = np.stack([res.results[b]["out"] for b in range(B)])
    return out.astype(np.float32)



# revision 17
# speedup vs baseline: 1.0912x; 1.0912x over previous
"""Fused multi-head causal+padding attention for Trainium2 (Bass/Tile).

Problem: nn_Attention (B=8, T=1024, C=512, H=8, D=64, TT=4), f32.
Sharding: data-parallel over batch B across 8 NeuronCores (1 batch elem/core).

v2: exploits causal sparsity. The mask is m[q,k] = (k<=q) | padding[q], i.e.
causal except ~10% of q-columns ("pad columns") are fully open. Split:
  - causal pass: scores/exp/AV only over the live region q >= 128*kt per
    k-tile (56% of the full T x T work), head-PAIR score matmuls packed into
    the PE via K=64 row groups (2 heads per pass).
  - pad side path: gather the pad columns of q via a selection matmul on x,
    project to q_pad [d, PP], compute full-column scores/exp/AV for just
    those PP=128 columns (bf16), mask to the strictly-below-diagonal k-tiles,
    and scatter-add the results into each head's PSUM accumulator with a
    one-hot matmul.
Denominators ride along as an appended ones-column on v (row 64 of y_ps).

Per-core pipeline otherwise as v1: x^T via PE transposes; qk^T = W_qk^T@x^T;
v = x@W_v; per-column normalize via reciprocal + partition_broadcast;
out = y @ W_proj with bias preloaded into PSUM via a K=1 matmul.

Host-side prep folds 1/sqrt(D) into W_q/b_q, folds b_v@W_proj+b_proj into one
output bias, pre-rounds x/weights to tf32 (f32r), and builds the mask/select
tensors (mdiag, mdead, sel, selpT) from padding_mask.
"""

import numpy as np
import ml_dtypes
from contextlib import ExitStack

B, T, C, H, TT = 8, 1024, 512, 8, 4
D = C // H
NCORES = 8
TK = T // 128   # 8 tiles of 128 along t
CK = C // 128   # 4 tiles of 128 along c
PP = 128        # pad columns, padded to 128

_CACHE = {}


def _chunks(kt):
    """Live q chunks [128*kt, T) for k-tile kt, split at 512 (PSUM banks)."""
    s = 128 * kt
    out = []
    while s < T:
        e = min(T, (s // 512 + 1) * 512)
        out.append((s, e - s))
        s = e
    return out


def _build_nc(reps=1, upto=4, debug=False):
    import concourse.bass as bass
    import concourse.mybir as mybir
    import concourse.tile as tile
    from concourse import bacc
    from concourse.bass import ts
    from concourse.masks import make_identity

    dt = mybir.dt
    F32, F32R, BF16 = dt.float32, dt.float32r, dt.bfloat16
    AF = mybir.ActivationFunctionType

    nc = bacc.Bacc("TRN2", target_bir_lowering=False, debug=False,
                   num_devices=NCORES)

    x_d = nc.dram_tensor("x", [T, C], F32R, kind="ExternalInput").ap()
    wqk_d = nc.dram_tensor("wqk", [C, 2 * C], F32R, kind="ExternalInput").ap()
    wv_d = nc.dram_tensor("wv", [C, C], F32R, kind="ExternalInput").ap()
    wp_d = nc.dram_tensor("wp", [C, C], F32R, kind="ExternalInput").ap()
    bqk_d = nc.dram_tensor("bqk", [2 * C], F32, kind="ExternalInput").ap()
    beff_d = nc.dram_tensor("beff", [1, C], F32R, kind="ExternalInput").ap()
    ones_d = nc.dram_tensor("ones1", [1, 128], F32R, kind="ExternalInput").ap()
    mdiag_d = nc.dram_tensor("mdiag", [128, TK * 128], BF16,
                             kind="ExternalInput").ap()
    sel_d = nc.dram_tensor("sel", [128, TK * PP], F32R,
                           kind="ExternalInput").ap()
    selpt_d = nc.dram_tensor("selpT", [128, T], BF16,
                             kind="ExternalInput").ap()
    mdead_d = nc.dram_tensor("mdead", [128, TK * PP], BF16,
                             kind="ExternalInput").ap()
    out_d = nc.dram_tensor("out", [T, C], F32, kind="ExternalOutput").ap()
    dbg = {}
    if debug:
        dbg["xpad"] = nc.dram_tensor("dbg_xpad", [128, C], F32R,
                                     kind="ExternalOutput").ap()
        dbg["qkT"] = nc.dram_tensor("dbg_qkT", [128, 8 * T], F32R,
                                    kind="ExternalOutput").ap()
        dbg["qpad"] = nc.dram_tensor("dbg_qpad", [128, CK * PP], BF16,
                                     kind="ExternalOutput").ap()
        dbg["ypadT"] = nc.dram_tensor("dbg_ypadT", [128, H * (D + 1)], BF16,
                                      kind="ExternalOutput").ap()
        dbg["yT"] = nc.dram_tensor("dbg_yT", [128, CK * T], F32R,
                                   kind="ExternalOutput").ap()
        dbg["et"] = nc.dram_tensor("dbg_et", [128, 2 * T], BF16,
                                   kind="ExternalOutput").ap()
        dbg["yps"] = nc.dram_tensor("dbg_yps", [D + 1, T], F32,
                                    kind="ExternalOutput").ap()

    with tile.TileContext(nc) as tc, ExitStack() as ctx:
        consts = ctx.enter_context(tc.tile_pool(name="consts", bufs=1))

        ident_f = consts.tile([128, 128], F32)
        make_identity(nc, ident_f)
        ident = consts.tile([128, 128], F32R)
        nc.vector.tensor_copy(ident, ident_f)
        # dummy exp: pulls the ~2.7us ACT exp-table load into the DMA-bound
        # front (ACT idle here) instead of the first real exp in attention
        warm = consts.tile([1, 128], F32)
        nc.scalar.activation(warm, ident[0:1, :], AF.Exp)
        # mask/select tensors on SWDGE; sel first (needed in phase 1)
        sel_s = consts.tile([128, TK, PP], F32R)
        nc.gpsimd.dma_start(out=sel_s, in_=sel_d.rearrange("p (t j) -> p t j", t=TK))
        bqk_s = consts.tile([128, 2 * C // 128], F32)
        nc.gpsimd.dma_start(out=bqk_s, in_=bqk_d.rearrange("(i p) -> p i", p=128))
        mdiag_s = consts.tile([128, TK, 128], BF16)
        nc.gpsimd.dma_start(out=mdiag_s, in_=mdiag_d.rearrange("p (t q) -> p t q", t=TK))
        mdead_s = consts.tile([128, TK, PP], BF16)
        nc.gpsimd.dma_start(out=mdead_s, in_=mdead_d.rearrange("p (t j) -> p t j", t=TK))
        selpt_s = consts.tile([128, 2, 512], BF16)
        nc.gpsimd.dma_start(out=selpt_s, in_=selpt_d.rearrange("p (h q) -> p h q", h=2))
        # weights on the scalar-engine HWDGE queue so the x loads (sync
        # queue) land first and compute starts immediately
        wqk_s = consts.tile([128, CK, 2 * C], F32R)
        nc.scalar.dma_start(out=wqk_s, in_=wqk_d.rearrange("(j p) n -> p j n", p=128))
        wv_s = consts.tile([128, CK, C], F32R)
        nc.scalar.dma_start(out=wv_s, in_=wv_d.rearrange("(j p) n -> p j n", p=128))
        wp_s = consts.tile([128, CK, C], F32R)
        nc.scalar.dma_start(out=wp_s, in_=wp_d.rearrange("(j p) n -> p j n", p=128))
        beff_s = consts.tile([1, C], F32R)
        nc.scalar.dma_start(out=beff_s, in_=beff_d)
        ones1 = consts.tile([1, 128], F32R)
        nc.scalar.dma_start(out=ones1, in_=ones_d)

        xT = consts.tile([128, CK, T], F32R)
        qkT = consts.tile([128, 2 * C // 128, T], F32R)
        kTb = consts.tile([128, CK, T], BF16)
        vaug = consts.tile([128, TK, H, D + 1], BF16)
        yT = consts.tile([128, CK, T], F32R)
        x_pad = consts.tile([128, C], F32R)
        x_padT = consts.tile([128, CK, PP], F32R)
        q_padT = consts.tile([128, C], F32R)
        q_pad = consts.tile([128, CK, PP], BF16)
        y_padT = consts.tile([128, H, D + 1], BF16)

        def run_body():
            body(nc, tc, ts, F32, F32R, BF16, AF,
                 x_d, out_d, ident, wqk_s, wv_s, wp_s, bqk_s, beff_s, ones1,
                 sel_s, mdiag_s, mdead_s, selpt_s,
                 xT, qkT, kTb, vaug, yT,
                 x_pad, x_padT, q_padT, q_pad, y_padT, upto,
                 dbg if debug else None)
            if debug:
                nc.sync.dma_start(out=dbg["xpad"], in_=x_pad)
                nc.sync.dma_start(
                    out=dbg["qkT"].rearrange("p (i t) -> p i t", i=8), in_=qkT)
                nc.sync.dma_start(
                    out=dbg["qpad"].rearrange("p (i j) -> p i j", i=CK),
                    in_=q_pad)
                nc.sync.dma_start(
                    out=dbg["ypadT"].rearrange("p (h d) -> p h d", h=H),
                    in_=y_padT)
                nc.sync.dma_start(
                    out=dbg["yT"].rearrange("p (i t) -> p i t", i=CK), in_=yT)

        if reps == 1:
            run_body()
        else:
            with tc.For_i(0, reps, 1):
                run_body()

    nc.compile()
    return nc


def body(nc, tc, ts, F32, F32R, BF16, AF,
         x_d, out_d, ident, wqk_s, wv_s, wp_s, bqk_s, beff_s, ones1,
         sel_s, mdiag_s, mdead_s, selpt_s,
         xT, qkT, kTb, vaug, yT,
         x_pad, x_padT, q_padT, q_pad, y_padT, upto=4, dbg=None):
        # ---- phase 1: load x, transpose to x^T; select pad rows of x ----
        with tc.tile_pool(name="xstage", bufs=3) as xst, \
             tc.tile_pool(name="ptr", bufs=4, space="PSUM") as ptr, \
             tc.tile_pool(name="xselp", bufs=1, space="PSUM") as xselp:
            x_pad_ps = xselp.tile([128, C], F32)
            for i in range(TK):
                xs = xst.tile([128, C], F32R)
                nc.sync.dma_start(out=xs, in_=x_d[ts(i, 128), :])
                for j in range(CK):
                    pt = ptr.tile([128, 128], F32R)
                    nc.tensor.transpose(pt, xs[:, ts(j, 128)], ident)
                    nc.vector.tensor_copy(xT[:, j, ts(i, 128)], pt)
                nc.tensor.matmul(x_pad_ps, sel_s[:, i, :], xs,
                                 start=(i == 0), stop=(i == TK - 1))
            nc.vector.tensor_copy(x_pad, x_pad_ps)

        # ---- pad-q prep: x_padT = T(x_pad); q_padT = x_pad @ W_q;
        #      q_pad = T(q_padT) + b_q  (bf16, [d, j] layout) ----
        with tc.tile_pool(name="ptr2", bufs=2, space="PSUM") as ptr2, \
             tc.tile_pool(name="qselp", bufs=1, space="PSUM") as qselp:
            for j in range(CK):
                pt = ptr2.tile([128, 128], F32R)
                nc.tensor.transpose(pt, x_pad[:, ts(j, 128)], ident)
                nc.vector.tensor_copy(x_padT[:, j, :], pt)
            q_padT_ps = qselp.tile([128, C], F32)
            for j in range(CK):
                nc.tensor.matmul(q_padT_ps, x_padT[:, j, :], wqk_s[:, j, 0:C],
                                 start=(j == 0), stop=(j == CK - 1))
            nc.vector.tensor_copy(q_padT, q_padT_ps)
            for i in range(CK):
                pt = ptr2.tile([128, 128], F32R)
                nc.tensor.transpose(pt, q_padT[:, ts(i, 128)], ident)
                nc.vector.tensor_scalar_add(q_pad[:, i, :], pt,
                                            bqk_s[:, i:i + 1])

        # ---- phase 2: qk^T (transposed) and v (standard, with ones col) ----
        nc.gpsimd.memset(vaug[:, :, :, D:D + 1], 1.0)
        with tc.tile_pool(name="pqk", bufs=4, space="PSUM") as pqk:

            def qk_tile(i):
                for n in range(T // 512):          # 512-chunk of t
                    ps = pqk.tile([128, 512], F32)
                    for j in range(CK):
                        nc.tensor.matmul(
                            ps, wqk_s[:, j, ts(i, 128)],
                            xT[:, j, ts(n, 512)],
                            start=(j == 0), stop=(j == CK - 1))
                    nc.vector.tensor_scalar_add(qkT[:, i, ts(n, 512)], ps,
                                                bqk_s[:, i:i + 1])
                    if i >= 4:
                        nc.vector.tensor_scalar_add(
                            kTb[:, i - 4, ts(n, 512)], ps, bqk_s[:, i:i + 1])

            def v_tile(i):
                ps = pqk.tile([128, 512], F32)
                for j in range(CK):
                    nc.tensor.matmul(
                        ps, xT[:, j, ts(i, 128)], wv_s[:, j, :],
                        start=(j == 0), stop=(j == CK - 1))
                nc.scalar.activation(
                    vaug[:, i, :, 0:D],
                    ps.rearrange("p (h d) -> p h d", h=H), AF.Copy)

            qk_tile(0)
            qk_tile(4)
            for i in range(TK):
                v_tile(i)
            for m in range(1, 4):
                qk_tile(m)
                qk_tile(4 + m)

        # ---- phase 3: attention ----
        if upto < 3:
            return
        with tc.tile_pool(name="expp", bufs=4) as expp, \
             tc.tile_pool(name="rp", bufs=2) as rp, \
             tc.tile_pool(name="rbp", bufs=2) as rbp:

            # pad side path for all 4 head pairs: scores for the PP pad
            # columns over all k (bf16, head pairs packed via K=64 row
            # groups), exp, dead-zone mask, AV^T, evacuate to SBUF bf16.
            with tc.tile_pool(name="ps_sp", bufs=1, space="PSUM") as ps_sp, \
                 tc.tile_pool(name="ps_yp", bufs=2, space="PSUM") as ps_yp:
                for m in range(4):
                    s0 = ps_sp.tile([128, TK, PP], F32)
                    s1 = ps_sp.tile([128, TK, PP], F32)
                    for kt in range(TK):
                        nc.tensor.matmul(s0[:, kt, :], kTb[0:64, m, ts(kt, 128)],
                                         q_pad[0:64, m, :], start=True, stop=True)
                        nc.tensor.matmul(s1[:, kt, :], kTb[64:128, m, ts(kt, 128)],
                                         q_pad[64:128, m, :], start=True, stop=True)
                    for h, s in ((2 * m, s0), (2 * m + 1, s1)):
                        et = expp.tile([128, TK, PP], BF16)
                        nc.scalar.activation(et, s, AF.Exp)
                        nc.gpsimd.tensor_mul(et, et, mdead_s)
                        ypp = ps_yp.tile([128, D + 1], F32)
                        for kt in range(TK):
                            nc.tensor.matmul(ypp, et[:, kt, :],
                                             vaug[:, kt, h, :],
                                             start=(kt == 0), stop=(kt == TK - 1))
                        nc.vector.tensor_copy(y_padT[:, h, :], ypp)

            ps_s_cm = tc.tile_pool(name="ps_s", bufs=1, space="PSUM")
            ps_s = ps_s_cm.__enter__()
            ps_y_cm = tc.tile_pool(name="ps_y", bufs=1, space="PSUM")
            ps_y = ps_y_cm.__enter__()

            # causal pass per head pair; AV for k-tile kt is emitted after
            # the scores for k-tile kt+1 so the PE never waits on exp.
            for m in range(4):
                h0, h1 = 2 * m, 2 * m + 1
                qt, kt_ = m, 4 + m
                y0 = ps_y.tile([D + 1, T], F32)
                y1 = ps_y.tile([D + 1, T], F32)

                def emit_av(kt, et0, et1):
                    for yps, et, h in ((y0, et0, h0), (y1, et1, h1)):
                        for (qs, w) in _chunks(kt):
                            nc.tensor.matmul(
                                yps[:, qs:qs + w], vaug[:, kt, h, :],
                                et[:, qs:qs + w],
                                start=(kt == 0), stop=False)

                pend = None
                for kt in range(TK):
                    s0 = ps_s.tile([128, T], F32)
                    s1 = ps_s.tile([128, T], F32)
                    for (qs, w) in _chunks(kt):
                        nc.tensor.matmul(
                            s0[:, qs:qs + w], qkT[0:64, kt_, ts(kt, 128)],
                            qkT[0:64, qt, qs:qs + w], start=True, stop=True)
                        nc.tensor.matmul(
                            s1[:, qs:qs + w], qkT[64:128, kt_, ts(kt, 128)],
                            qkT[64:128, qt, qs:qs + w], start=True, stop=True)
                    lo = 128 * kt
                    et0 = expp.tile([128, T], BF16)
                    et1 = expp.tile([128, T], BF16)
                    nc.scalar.activation(et0[:, lo:], s0[:, lo:], AF.Exp)
                    nc.vector.tensor_mul(et0[:, lo:lo + 128],
                                         et0[:, lo:lo + 128],
                                         mdiag_s[:, kt, :])
                    nc.scalar.activation(et1[:, lo:], s1[:, lo:], AF.Exp)
                    nc.vector.tensor_mul(et1[:, lo:lo + 128],
                                         et1[:, lo:lo + 128],
                                         mdiag_s[:, kt, :])
                    if dbg is not None and m == 0 and kt in (0, 2):
                        slot = 0 if kt == 0 else 1
                        nc.sync.dma_start(
                            out=dbg["et"][:, slot * T + lo:(slot + 1) * T],
                            in_=et0[:, lo:])
                    if pend is not None:
                        emit_av(*pend)
                    pend = (kt, et0, et1)
                emit_av(*pend)

                # scatter pad-column results into the accumulators, closing
                # each PSUM accumulation group
                for yps, h in ((y0, h0), (y1, h1)):
                    for qh in range(2):
                        nc.tensor.matmul(yps[:, ts(qh, 512)],
                                         y_padT[:, h, :], selpt_s[:, qh, :],
                                         start=False, stop=True)

                if dbg is not None and m == 0:
                    yps_s, _free = tc.tile([D + 1, T], F32,
                                           name="dbg_yps_s")
                    nc.vector.tensor_copy(yps_s, y0)
                    nc.sync.dma_start(out=dbg["yps"], in_=yps_s)
                    _free()

                # normalize: yT[d, q] = y[d, q] / den[q]
                for yps, h in ((y0, h0), (y1, h1)):
                    po = (h % 2) * 64
                    rec = rp.tile([1, T], F32)
                    nc.vector.reciprocal(rec, yps[D:D + 1, :])
                    rb = rbp.tile([D, T], F32)
                    nc.gpsimd.partition_broadcast(rb, rec)
                    nc.vector.tensor_mul(yT[po:po + 64, qt, :],
                                         yps[0:D, :], rb)

            ps_y_cm.__exit__(None, None, None)
            ps_s_cm.__exit__(None, None, None)

        # ---- phase 4: out = y @ W_proj + b_eff ----
        if upto < 4:
            return
        with tc.tile_pool(name="pp", bufs=2, space="PSUM") as pp, \
             tc.tile_pool(name="outst", bufs=3) as outst:
            for i in range(TK):
                ps = pp.tile([128, C], F32)
                nc.tensor.matmul(ps, ones1, beff_s,
                                 start=True, stop=False)
                for j in range(CK):
                    nc.tensor.matmul(ps, yT[:, j, ts(i, 128)],
                                     wp_s[:, j, :],
                                     start=False, stop=(j == CK - 1))
                ot = outst.tile([128, C], F32)
                nc.scalar.copy(ot, ps)
                nc.sync.dma_start(out=out_d[ts(i, 128), :], in_=ot)


def get_nc(reps=1, upto=4, debug=False):
    key = ("nc", reps, upto, debug)
    if key not in _CACHE:
        _CACHE[key] = _build_nc(reps, upto, debug)
    return _CACHE[key]


def tf32_round(a):
    """Round-to-nearest-even to tf32 (10-bit mantissa). fp32r operands must be
    pre-rounded: the BIR verifier requires every producer of fp32r-matmul
    operands to emit rounded values, and DMA can't convert."""
    a = np.ascontiguousarray(a, np.float32)
    b = a.view(np.uint32)
    lsb = (b >> np.uint32(13)) & np.uint32(1)
    r = b + np.uint32(0x0FFF) + lsb
    return ((r >> np.uint32(13)) << np.uint32(13)).view(np.float32)


def make_in_maps(x, padding_mask, W_qkv, b_qkv, W_proj, b_proj):
    x = np.asarray(x, np.float32)
    padding_mask = np.asarray(padding_mask, bool)
    W_qkv = np.asarray(W_qkv, np.float32)
    b_qkv = np.asarray(b_qkv, np.float32)
    W_proj = np.asarray(W_proj, np.float32)
    b_proj = np.asarray(b_proj, np.float32)

    scale = np.float32(1.0 / np.sqrt(D))
    wqk = np.concatenate([W_qkv[:, :C] * scale, W_qkv[:, C:2 * C]], axis=1)
    wqk = tf32_round(wqk)
    wv = tf32_round(W_qkv[:, 2 * C:])
    wp = tf32_round(W_proj)
    bqk = np.concatenate([b_qkv[:C] * scale, b_qkv[C:2 * C]]).astype(np.float32)
    beff = tf32_round((b_qkv[2 * C:] @ W_proj + b_proj).reshape(1, C))

    # per-group mask/select tensors
    tri = (np.arange(128)[:, None] <= np.arange(128)[None, :])
    grp = {}
    for g in range(TT):
        p = padding_mask[g]
        idx = np.nonzero(p)[0]
        P = len(idx)
        assert P <= PP, f"group {g}: {P} pad columns > {PP}"
        mdiag = np.zeros((128, TK, 128), np.float32)
        for kt in range(TK):
            mdiag[:, kt, :] = tri | p[kt * 128:(kt + 1) * 128][None, :]
        sel = np.zeros((128, TK, PP), np.float32)
        selpT = np.zeros((128, 2, 512), np.float32)
        m2 = np.zeros((TK, PP), np.float32)
        for j, q in enumerate(idx):
            sel[q % 128, q // 128, j] = 1.0
            selpT[j, q // 512, q % 512] = 1.0
            m2[:, j] = (np.arange(TK) > q // 128)
        mdead = np.broadcast_to(m2[None], (128, TK, PP))
        grp[g] = {
            "mdiag": np.ascontiguousarray(
                mdiag.reshape(128, TK * 128).astype(ml_dtypes.bfloat16)),
            "sel": np.ascontiguousarray(sel.reshape(128, TK * PP)),
            "selpT": np.ascontiguousarray(
                selpT.reshape(128, 2 * 512).astype(ml_dtypes.bfloat16)),
            "mdead": np.ascontiguousarray(
                mdead.reshape(128, TK * PP).astype(ml_dtypes.bfloat16)),
        }

    in_maps = []
    for b in range(B):
        g = b % TT
        in_maps.append({
            "x": tf32_round(x[b]),
            "wqk": wqk, "wv": wv, "wp": wp,
            "bqk": bqk, "beff": beff,
            "ones1": np.ones((1, 128), np.float32),
            "mdiag": grp[g]["mdiag"], "sel": grp[g]["sel"],
            "selpT": grp[g]["selpT"], "mdead": grp[g]["mdead"],
        })
    return in_maps


def kernel(x, padding_mask, W_qkv, b_qkv, W_proj, b_proj):
    from concourse.bass_utils import run_bass_kernel_spmd

    nc = get_nc()
    in_maps = make_in_maps(x, padding_mask, W_qkv, b_qkv, W_proj, b_proj)
    res = run_bass_kernel_spmd(nc, in_maps, list(range(NCORES)))
    out = np.stack([res.results[b]["out"] for b in range(B)])
    return out.astype(np.float32)
